# revision 19
# baseline (speedup 1.0000x reference)
"""Trainium2 Bass kernel for nn_EnergyDistributionCNN (3x3 conv -> unfold ->
softmax over patch -> weighted -> fold overlap-add), 8 NeuronCores.

Math (algebraically identical to the torch/jax reference):
    out = conv3x3(x, k)            cross-correlation, zero pad 1
    E   = exp(out)
    Z   = boxsum3x3(E padded with ONES)   (zero pads contribute exp(0)=1)
    U   = x / Z
    S   = boxsum3x3(U zero-padded)
    result = E * S

Sharding: row-block across 8 cores with a 3-row halo sliced on the host
(zero-filled at the global edges) -- no device-to-device communication.
Global boundary rows are handled uniformly by a per-row mask fused into the
exp's per-partition scale (exp(0*out)=1); boundary columns by host zero
padding plus static edge memsets.

On-core layout: rows on partitions, cols on the free dim, processed in
row-tiles (<=122 output rows) x width-halves. All vertical stencil mixing
runs on the TensorEngine via banded matrices; horizontal mixing is 3
column-shifted matmuls accumulated in PSUM. Everything on the PE uses
fp32r (full-rate moving operand, ~11-bit mantissa); x is fed to the PE by
bitcasting the f32 tile to f32r (the PE rounds internally; the resulting
~1e-3 conv error is well inside the 2e-2 gate).
exp runs on the ScalarEngine directly from conv's PSUM; 1/Z uses the DVE
fast reciprocal (~18 bits). Band row-mappings put every compute op at
partition base 0; the valid output rows sit at partitions [2, R+2), which
the (partition-unrestricted) output DMA reads.
"""

from contextlib import ExitStack

import numpy as np

import concourse.bacc as bacc
import concourse.mybir as mybir
import concourse.tile as tile
from concourse._compat import with_exitstack
from concourse.bass_utils import run_bass_kernel_spmd

F32 = mybir.dt.float32
F32R = mybir.dt.float32r

H = 4096
W = 4096
N_CORES = 8
RC = H // N_CORES  # rows per core
HALO = 3
RT = 122   # output rows per row-tile (RT + 6 <= 128 partitions)
WS = 2     # width splits (SBUF capacity)
WH = W // WS
C = 512    # matmul column chunk = one fp32 PSUM bank
NBUFS = 3
PS_BUFS = 3


# ---------------------------------------------------------------- host side

def _make_bands(k: np.ndarray) -> np.ndarray:
    """bands[v][p, m] = k[p-m, v] (conv, v=0..2); bands[3] = BB ones with
    p-m in 0..2 (S matmul); bands[4] = BT ones with m-p in 0..2 (Z).
    bands[5..9]: same five patterns as 4x block-diagonal 32x32 blocks, for
    the column-folded last row-tile."""
    bands = np.zeros((10, 128, 128), np.float32)
    idx = np.arange(128)
    for d in range(3):
        p = idx[d:]
        m = idx[: 128 - d]
        for v in range(3):
            bands[v, p, m] = k[d, v]
        bands[3, p, m] = 1.0
        bands[4, m, p] = 1.0
    for i in range(5):
        blk = bands[i][:32, :32]
        for b in range(4):
            bands[5 + i][32 * b : 32 * b + 32, 32 * b : 32 * b + 32] = blk
    return bands


def _make_core_inputs(x: np.ndarray, bands: np.ndarray, core: int):
    r0 = core * RC
    lo, hi = r0 - HALO, r0 + RC + HALO
    # 2 extra zero rows let the folded last tile load full 32-row blocks
    xh = np.zeros((RC + 2 * HALO + 2, W + 2 * HALO), np.float32)
    s_lo, s_hi = max(lo, 0), min(hi, H)
    xh[s_lo - lo : s_hi - lo, HALO : HALO + W] = x[s_lo:s_hi]
    gl = np.arange(lo, hi)
    mask = ((gl >= 0) & (gl < H)).astype(np.float32)[:, None]
    # folded-tile mask: 4 stacked 32-row blocks, rows [of+1, of+Rf+5) each
    of = (RC // RT) * RT
    Rf = RC - of
    maskf = np.zeros((128, 1), np.float32)
    for b in range(4):
        maskf[32 * b : 32 * b + Rf + 4] = mask[of + 1 : of + Rf + 5]
    return {"xh": xh, "mask": mask, "maskf": maskf, "bands": bands}


def _make_tiles():
    tiles = []
    o = 0
    while o < RC:
        R = min(RT, RC - o)
        tiles.append((o, R))
        o += R
    return tiles


def _chunks(total: int, width: int = C):
    out = []
    s = 0
    while s < total:
        out.append((s, min(width, total - s)))
        s += width
    return out


def _zchunks(total: int):
    """1024-col chunks (2 PSUM banks) with a small tail."""
    out = []
    s = 0
    while s < total:
        out.append((s, min(2 * C, total - s)))
        s += 2 * C
    return out


def _hchunks(total: int):
    """Split into ceil-even parts of ~1024 (SBUF ops, no bank limit)."""
    n = max(1, (total + C) // (2 * C))
    base, rem = divmod(total, n)
    out = []
    s = 0
    for i in range(n):
        cl = base + (1 if i < rem else 0)
        out.append((s, cl))
        s += cl
    return out


# -------------------------------------------------------------- device side

@with_exitstack
def _energy_body(ctx: ExitStack, tc, out_d, xh_d, mask_d, maskf_d, bands_d):
    nc = tc.nc
    Exp = mybir.ActivationFunctionType.Exp

    # ---- constants: band matrices DMA'd straight to f32r SBUF (byte copy);
    # the folded set is its own DMA, first -- the first emitted unit needs it
    consts = ctx.enter_context(tc.tile_pool(name="consts", bufs=1))
    bigb = consts.tile([128, 10 * 128], F32R, name="bigb")
    bands_r = bands_d.bitcast(F32R)
    nc.sync.dma_start(
        out=bigb[:, : 5 * 128].rearrange("p (i m) -> p i m", i=5),
        in_=bands_r[0:5].rearrange("i p m -> p i m"),
    )

    def band_views(base):
        mh = [bigb[:, (base + v) * 128 : (base + v + 1) * 128] for v in range(3)]
        bb = bigb[:, (base + 3) * 128 : (base + 4) * 128]
        bt = bigb[:, (base + 4) * 128 : (base + 5) * 128]
        return mh, bb, bt

    MhiF, BBF, BTF = band_views(5)
    Mhi, BB, BT = band_views(0)
    SEGW = WH // 4

    xpool = ctx.enter_context(tc.tile_pool(name="xp", bufs=3))
    epool = ctx.enter_context(tc.tile_pool(name="ep", bufs=5))
    upool = ctx.enter_context(tc.tile_pool(name="up", bufs=5))
    rzpool = ctx.enter_context(tc.tile_pool(name="rzp", bufs=3))
    apool = ctx.enter_context(tc.tile_pool(name="ap", bufs=2))
    respool = ctx.enter_context(tc.tile_pool(name="resp", bufs=3))
    mpool = ctx.enter_context(tc.tile_pool(name="mp", bufs=2))
    ps_conv = ctx.enter_context(tc.tile_pool(name="psc", bufs=3, space="PSUM"))
    ps_z = ctx.enter_context(tc.tile_pool(name="psz", bufs=2, space="PSUM"))
    ps_s = ctx.enter_context(tc.tile_pool(name="pss", bufs=3, space="PSUM"))

    tiles = _make_tiles()

    class Unit:
        """One (row-tile, width-half) pipeline unit, emitted in 3 phases so
        every engine's in-order queue only ever waits on work from >=1 units
        earlier (no head-of-line blocking on the bottleneck PE):
          p1: X/mask DMA; conv (3 shifted fp32r band matmuls, PE) -> exp (Act)
          p2: Z (3 shifted BT matmuls, PE) -> 1/Z (DVE) -> U = x*Rz (GpSimd)
          p3: S (3 shifted BB matmuls on U, PE) -> res = E*S (DVE, PSUM in)
        """

        def __init__(self, kind, o, R, h):
            self.kind, self.o, self.R, self.h = kind, o, R, h
            self.Mh, self.bb, self.bt = (Mhi, BB, BT) if kind == "n" else (
                MhiF, BBF, BTF)
            self.rX, self.rE, self.rS = (
                (R + 6, R + 4, R + 2) if kind == "n" else (128, 128, 128))
            self.PW = WH if kind == "n" else SEGW
            self.g0 = h * WH
            self.CW = C

        def p1(self):
            o, R, h, g0 = self.o, self.R, self.h, self.g0
            mk = mpool.tile([128, 1], F32, tag="mk")
            if self.kind == "n":
                nc.sync.dma_start(out=mk[: R + 4], in_=mask_d[o + 1 : o + R + 5, :])
            else:
                nc.sync.dma_start(out=mk, in_=maskf_d)
            X = xpool.tile([128, WH + 6], F32R, tag="X")
            if self.kind == "n":
                hw2 = (WH + 6) // 2
                nc.sync.dma_start(
                    out=X[: self.rX, :hw2],
                    in_=xh_d[o : o + R + 6, g0 : g0 + hw2].bitcast(F32R),
                )
                nc.sync.dma_start(
                    out=X[: self.rX, hw2:],
                    in_=xh_d[o : o + R + 6, g0 + hw2 : g0 + WH + 6].bitcast(F32R),
                )
            else:
                for b in range(4):
                    nc.sync.dma_start(
                        out=X[32 * b : 32 * b + 32, : SEGW + 6],
                        in_=xh_d[
                            o : o + 32, g0 + b * SEGW : g0 + b * SEGW + SEGW + 6
                        ].bitcast(F32R),
                    )
            self.X, self.mk = X, mk

            rX, rE = self.rX, self.rE
            E = epool.tile([128, WH + 4], F32R, tag="E")
            ch = _chunks(self.PW + 4, self.CW)
            for idx, (cs, cl) in enumerate(ch):
                pc = ps_conv.tile([128, C], F32, tag="pc")
                for v in range(3):
                    nc.tensor.matmul(
                        pc[:rE, :cl],
                        self.Mh[v][:rX, :rE],
                        X[:rX, cs + v : cs + v + cl],
                        start=(v == 0),
                        stop=(v == 2),
                    )
                nc.scalar.activation(E[:rE, cs : cs + cl], pc[:rE, :cl], Exp, scale=mk[:rE])
                # global-edge cols of E are pad pixels: overwrite with
                # exp(0*x) = 1 on the same engine (no cross-engine dep)
                if idx == 0 and h == 0:
                    ep = E[:rE, 0:2] if self.kind == "n" else E[0:32, 0:2]
                    pp = pc[:rE, 0:2] if self.kind == "n" else pc[0:32, 0:2]
                    nc.scalar.activation(ep, pp, Exp, scale=0.0)
                if idx == len(ch) - 1 and h == WS - 1:
                    w = self.PW
                    ep = (E[:rE, w + 2 : w + 4] if self.kind == "n"
                          else E[96:128, w + 2 : w + 4])
                    pp = (pc[:rE, cl - 2 : cl] if self.kind == "n"
                          else pc[96:128, cl - 2 : cl])
                    nc.scalar.activation(ep, pp, Exp, scale=0.0)
            self.E = E

        def p2(self):
            rE, h = self.rE, self.h
            E, X = self.E, self.X
            # A1 = E0 + E1 (horizontal pre-tap), chunks alternating DVE/GpSimd;
            # Z then needs only 2 shifted matmuls: BT@A1 + BT@E2
            A1 = apool.tile([128, WH + 3], F32R, tag="A1")
            U = upool.tile([128, WH + 2], F32R, tag="U")
            wA = self.PW + 3
            ch = _chunks(self.PW + 2, self.CW)
            for idx, (cs, cl) in enumerate(ch):
                al = min(cl + (1 if idx == len(ch) - 1 else 0), wA - cs)
                eng = nc.vector if idx % 2 == 0 else nc.gpsimd
                eng.tensor_add(
                    out=A1[:rE, cs : cs + al],
                    in0=E[:rE, cs : cs + al],
                    in1=E[:rE, cs + 1 : cs + 1 + al],
                )
                pz = ps_z.tile([128, C], F32, tag="pz")
                nc.tensor.matmul(
                    pz[:rE, :cl],
                    self.bt[:rE, :rE],
                    A1[:rE, cs : cs + cl],
                    start=True,
                    stop=False,
                )
                nc.tensor.matmul(
                    pz[:rE, :cl],
                    self.bt[:rE, :rE],
                    E[:rE, cs + 2 : cs + 2 + cl],
                    start=False,
                    stop=True,
                )
                Rz = rzpool.tile([128, C], F32, tag="Rz")
                nc.vector.reciprocal_approx_fast(out=Rz[:rE, :cl], in_=pz[:rE, :cl])
                nc.gpsimd.tensor_mul(
                    out=U[:rE, cs : cs + cl],
                    in0=X[:rE, cs + 2 : cs + 2 + cl],
                    in1=Rz[:rE, :cl],
                )
                # U at global-edge pad columns is 0 (fold drops OOB)
                if idx == 0 and h == 0:
                    up = U[:rE, 0:1] if self.kind == "n" else U[0:32, 0:1]
                    nc.gpsimd.memset(up.bitcast(F32), 0.0)
                if idx == len(ch) - 1 and h == WS - 1:
                    w = self.PW
                    up = (U[:rE, w + 1 : w + 2] if self.kind == "n"
                          else U[96:128, w + 1 : w + 2])
                    nc.gpsimd.memset(up.bitcast(F32), 0.0)
            self.U = U

        def p3(self):
            o, R, g0 = self.o, self.R, self.g0
            rE, rS = self.rE, self.rS
            E, U = self.E, self.U
            res = respool.tile([128, WH], F32, tag="res")
            for cs, cl in _chunks(self.PW, self.CW):
                ps = ps_s.tile([128, C], F32, tag="ps")
                for v in range(3):
                    nc.tensor.matmul(
                        ps[:rS, :cl],
                        self.bb[:rE, :rS],
                        U[:rE, cs + v : cs + v + cl],
                        start=(v == 0),
                        stop=(v == 2),
                    )
                nc.vector.tensor_mul(
                    out=res[:rS, cs : cs + cl],
                    in0=E[:rS, cs + 2 : cs + 2 + cl],
                    in1=ps[:rS, :cl],
                )
            if self.kind == "n":
                nc.sync.dma_start(
                    out=out_d[o : o + R, g0 : g0 + WH], in_=res[2 : R + 2, :WH]
                )
            else:
                for b in range(4):
                    nc.sync.dma_start(
                        out=out_d[o : o + R, g0 + b * SEGW : g0 + (b + 1) * SEGW],
                        in_=res[32 * b + 2 : 32 * b + 2 + R, :SEGW],
                    )

    of, Rf = tiles[-1]
    units = []
    for o, R in tiles[:-1]:
        for h in range(WS):
            units.append(Unit("n", o, R, h))
    units.insert(1, Unit("f", of, Rf, 0))
    units.append(Unit("f", of, Rf, WS - 1))
    units[-1].CW = 128  # short serial chain at the drain

    LAG2, LAG3 = 1, 2
    n = len(units)
    for i in range(n + LAG3):
        if i < n:
            units[i].p1()
        if i == 0:
            nc.sync.dma_start(
                out=bigb[:, 5 * 128 :].rearrange("p (i m) -> p i m", i=5),
                in_=bands_r[5:10].rearrange("i p m -> p i m"),
            )
        if 0 <= i - LAG2 < n:
            units[i - LAG2].p2()
        if 0 <= i - LAG3 < n:
            units[i - LAG3].p3()


_CACHE: dict = {}


def _build():
    if "nc" in _CACHE:
        return _CACHE["nc"]
    nc = bacc.Bacc(
        "TRN2", target_bir_lowering=False, debug=False, num_devices=N_CORES
    )
    xh_d = nc.dram_tensor(
        "xh", (RC + 2 * HALO + 2, W + 2 * HALO), F32, kind="ExternalInput"
    ).ap()
    mask_d = nc.dram_tensor("mask", (RC + 2 * HALO, 1), F32, kind="ExternalInput").ap()
    maskf_d = nc.dram_tensor("maskf", (128, 1), F32, kind="ExternalInput").ap()
    bands_d = nc.dram_tensor("bands", (10, 128, 128), F32, kind="ExternalInput").ap()
    out_d = nc.dram_tensor("out", (RC, W), F32, kind="ExternalOutput").ap()
    with tile.TileContext(nc) as tc:
        _energy_body(tc, out_d, xh_d, mask_d, maskf_d, bands_d)
    nc.compile()
    _CACHE["nc"] = nc
    return nc


def kernel(shareable_energy: np.ndarray, kernel: np.ndarray, **_run_kw) -> np.ndarray:
    x = np.ascontiguousarray(np.asarray(shareable_energy, np.float32))
    k = np.asarray(kernel, np.float32)
    assert x.shape == (H, W), x.shape
    nc = _build()
    bands = _make_bands(k)
    in_maps = [_make_core_inputs(x, bands, core) for core in range(N_CORES)]
    r = run_bass_kernel_spmd(nc, in_maps, core_ids=list(range(N_CORES)), **_run_kw)
    out = np.concatenate([res["out"] for res in r.results], axis=0)
    if _run_kw:
        _CACHE["last_result"] = r
    return out



# revision 20
# speedup vs baseline: 1.0370x; 1.0370x over previous
"""Trainium2 Bass kernel for nn_EnergyDistributionCNN (3x3 conv -> unfold ->
softmax over patch -> weighted -> fold overlap-add), 8 NeuronCores.

Math (algebraically identical to the torch/jax reference):
    out = conv3x3(x, k)            cross-correlation, zero pad 1
    E   = exp(out)
    Z   = boxsum3x3(E padded with ONES)   (zero pads contribute exp(0)=1)
    U   = x / Z
    S   = boxsum3x3(U zero-padded)
    result = E * S

Sharding: row-block across 8 cores with a 3-row halo sliced on the host
(zero-filled at the global edges) -- no device-to-device communication.
Global boundary rows are handled uniformly by a per-row mask fused into the
exp's per-partition scale (exp(0*out)=1); boundary columns by host zero
padding plus static edge memsets.

On-core layout: rows on partitions, cols on the free dim, processed in
row-tiles (<=122 output rows) x width-halves. All vertical stencil mixing
runs on the TensorEngine via banded matrices; horizontal mixing is 3
column-shifted matmuls accumulated in PSUM. Everything on the PE uses
fp32r (full-rate moving operand, ~11-bit mantissa); x is fed to the PE by
bitcasting the f32 tile to f32r (the PE rounds internally; the resulting
~1e-3 conv error is well inside the 2e-2 gate).
exp runs on the ScalarEngine directly from conv's PSUM; 1/Z uses the DVE
fast reciprocal (~18 bits). Band row-mappings put every compute op at
partition base 0; the valid output rows sit at partitions [2, R+2), which
the (partition-unrestricted) output DMA reads.
"""

from contextlib import ExitStack

import numpy as np

import concourse.bacc as bacc
import concourse.mybir as mybir
import concourse.tile as tile
from concourse._compat import with_exitstack
from concourse.bass_utils import run_bass_kernel_spmd

F32 = mybir.dt.float32
F32R = mybir.dt.float32r

H = 4096
W = 4096
N_CORES = 8
RC = H // N_CORES  # rows per core
HALO = 3
RT = 122   # output rows per row-tile (RT + 6 <= 128 partitions)
WS = 2     # width splits (SBUF capacity)
WH = W // WS
C = 512    # matmul column chunk = one fp32 PSUM bank
NBUFS = 3
PS_BUFS = 3


# ---------------------------------------------------------------- host side

def _make_bands(k: np.ndarray) -> np.ndarray:
    """bands[v][p, m] = k[p-m, v] (conv, v=0..2); bands[3] = BB ones with
    p-m in 0..2 (S matmul); bands[4] = BT ones with m-p in 0..2 (Z).
    bands[5..9]: same five patterns as 4x block-diagonal 32x32 blocks, for
    the column-folded last row-tile."""
    bands = np.zeros((10, 128, 128), np.float32)
    idx = np.arange(128)
    for d in range(3):
        p = idx[d:]
        m = idx[: 128 - d]
        for v in range(3):
            bands[v, p, m] = k[d, v]
        bands[3, p, m] = 1.0
        bands[4, m, p] = 1.0
    for i in range(5):
        blk = bands[i][:32, :32]
        for b in range(4):
            bands[5 + i][32 * b : 32 * b + 32, 32 * b : 32 * b + 32] = blk
    return bands


def _make_core_inputs(x: np.ndarray, bands: np.ndarray, core: int):
    r0 = core * RC
    lo, hi = r0 - HALO, r0 + RC + HALO
    # 2 extra zero rows let the folded last tile load full 32-row blocks
    xh = np.zeros((RC + 2 * HALO + 2, W + 2 * HALO), np.float32)
    s_lo, s_hi = max(lo, 0), min(hi, H)
    xh[s_lo - lo : s_hi - lo, HALO : HALO + W] = x[s_lo:s_hi]
    gl = np.arange(lo, hi)
    mask = ((gl >= 0) & (gl < H)).astype(np.float32)[:, None]
    # folded-tile mask: 4 stacked 32-row blocks, rows [of+1, of+Rf+5) each
    of = (RC // RT) * RT
    Rf = RC - of
    maskf = np.zeros((128, 1), np.float32)
    for b in range(4):
        maskf[32 * b : 32 * b + Rf + 4] = mask[of + 1 : of + Rf + 5]
    return {"xh": xh, "mask": mask, "maskf": maskf, "bands": bands}


def _make_tiles():
    tiles = []
    o = 0
    while o < RC:
        R = min(RT, RC - o)
        tiles.append((o, R))
        o += R
    return tiles


def _chunks(total: int, width: int = C):
    out = []
    s = 0
    while s < total:
        out.append((s, min(width, total - s)))
        s += width
    return out


def _zchunks(total: int):
    """1024-col chunks (2 PSUM banks) with a small tail."""
    out = []
    s = 0
    while s < total:
        out.append((s, min(2 * C, total - s)))
        s += 2 * C
    return out


def _hchunks(total: int):
    """Split into ceil-even parts of ~1024 (SBUF ops, no bank limit)."""
    n = max(1, (total + C) // (2 * C))
    base, rem = divmod(total, n)
    out = []
    s = 0
    for i in range(n):
        cl = base + (1 if i < rem else 0)
        out.append((s, cl))
        s += cl
    return out


# -------------------------------------------------------------- device side

@with_exitstack
def _energy_body(ctx: ExitStack, tc, out_d, xh_d, mask_d, maskf_d, bands_d):
    nc = tc.nc
    Exp = mybir.ActivationFunctionType.Exp

    # ---- constants: band matrices DMA'd straight to f32r SBUF (byte copy);
    # the folded set is its own DMA, first -- the first emitted unit needs it
    consts = ctx.enter_context(tc.tile_pool(name="consts", bufs=1))
    bigb = consts.tile([128, 10 * 128], F32R, name="bigb")
    bands_r = bands_d.bitcast(F32R)
    nc.sync.dma_start(
        out=bigb[:, : 5 * 128].rearrange("p (i m) -> p i m", i=5),
        in_=bands_r[0:5].rearrange("i p m -> p i m"),
    )

    def band_views(base):
        mh = [bigb[:, (base + v) * 128 : (base + v + 1) * 128] for v in range(3)]
        bb = bigb[:, (base + 3) * 128 : (base + 4) * 128]
        bt = bigb[:, (base + 4) * 128 : (base + 5) * 128]
        return mh, bb, bt

    MhiF, BBF, BTF = band_views(5)
    Mhi, BB, BT = band_views(0)
    SEGW = WH // 4

    xpool = ctx.enter_context(tc.tile_pool(name="xp", bufs=3))
    epool = ctx.enter_context(tc.tile_pool(name="ep", bufs=5))
    upool = ctx.enter_context(tc.tile_pool(name="up", bufs=5))
    rzpool = ctx.enter_context(tc.tile_pool(name="rzp", bufs=3))
    apool = ctx.enter_context(tc.tile_pool(name="ap", bufs=2))
    respool = ctx.enter_context(tc.tile_pool(name="resp", bufs=3))
    mpool = ctx.enter_context(tc.tile_pool(name="mp", bufs=2))
    ps_conv = ctx.enter_context(tc.tile_pool(name="psc", bufs=3, space="PSUM"))
    ps_z = ctx.enter_context(tc.tile_pool(name="psz", bufs=2, space="PSUM"))
    ps_s = ctx.enter_context(tc.tile_pool(name="pss", bufs=3, space="PSUM"))

    tiles = _make_tiles()

    class Unit:
        """One (row-tile, width-half) pipeline unit, emitted in 3 phases so
        every engine's in-order queue only ever waits on work from >=1 units
        earlier (no head-of-line blocking on the bottleneck PE):
          p1: X/mask DMA; conv (3 shifted fp32r band matmuls, PE) -> exp (Act)
          p2: Z (3 shifted BT matmuls, PE) -> 1/Z (DVE) -> U = x*Rz (GpSimd)
          p3: S (3 shifted BB matmuls on U, PE) -> res = E*S (DVE, PSUM in)
        """

        def __init__(self, kind, o, R, h):
            self.kind, self.o, self.R, self.h = kind, o, R, h
            self.Mh, self.bb, self.bt = (Mhi, BB, BT) if kind == "n" else (
                MhiF, BBF, BTF)
            self.rX, self.rE, self.rS = (
                (R + 6, R + 4, R + 2) if kind == "n" else (128, 128, 128))
            self.PW = WH if kind == "n" else SEGW
            self.g0 = h * WH
            self.CW = C

        def p1_dma(self):
            o, R, g0 = self.o, self.R, self.g0
            mk = mpool.tile([128, 1], F32, tag="mk")
            if self.kind == "n":
                nc.sync.dma_start(out=mk[: R + 4], in_=mask_d[o + 1 : o + R + 5, :])
            else:
                nc.sync.dma_start(out=mk, in_=maskf_d)
            X = xpool.tile([128, WH + 6], F32R, tag="X")
            if self.kind == "n":
                hw2 = (WH + 6) // 2
                nc.sync.dma_start(
                    out=X[: self.rX, :hw2],
                    in_=xh_d[o : o + R + 6, g0 : g0 + hw2].bitcast(F32R),
                )
                nc.sync.dma_start(
                    out=X[: self.rX, hw2:],
                    in_=xh_d[o : o + R + 6, g0 + hw2 : g0 + WH + 6].bitcast(F32R),
                )
            else:
                for b in range(4):
                    nc.sync.dma_start(
                        out=X[32 * b : 32 * b + 32, : SEGW + 6],
                        in_=xh_d[
                            o : o + 32, g0 + b * SEGW : g0 + b * SEGW + SEGW + 6
                        ].bitcast(F32R),
                    )
            self.X, self.mk = X, mk

        def p1(self):
            h = self.h
            X, mk = self.X, self.mk
            rX, rE = self.rX, self.rE
            E = epool.tile([128, WH + 4], F32R, tag="E")
            ch = _chunks(self.PW + 4, self.CW)
            for idx, (cs, cl) in enumerate(ch):
                pc = ps_conv.tile([128, C], F32, tag="pc")
                for v in range(3):
                    nc.tensor.matmul(
                        pc[:rE, :cl],
                        self.Mh[v][:rX, :rE],
                        X[:rX, cs + v : cs + v + cl],
                        start=(v == 0),
                        stop=(v == 2),
                    )
                nc.scalar.activation(E[:rE, cs : cs + cl], pc[:rE, :cl], Exp, scale=mk[:rE])
                # global-edge cols of E are pad pixels: overwrite with
                # exp(0*x) = 1 on the same engine (no cross-engine dep)
                if idx == 0 and h == 0:
                    ep = E[:rE, 0:2] if self.kind == "n" else E[0:32, 0:2]
                    pp = pc[:rE, 0:2] if self.kind == "n" else pc[0:32, 0:2]
                    nc.scalar.activation(ep, pp, Exp, scale=0.0)
                if idx == len(ch) - 1 and h == WS - 1:
                    w = self.PW
                    ep = (E[:rE, w + 2 : w + 4] if self.kind == "n"
                          else E[96:128, w + 2 : w + 4])
                    pp = (pc[:rE, cl - 2 : cl] if self.kind == "n"
                          else pc[96:128, cl - 2 : cl])
                    nc.scalar.activation(ep, pp, Exp, scale=0.0)
            self.E = E
            # A1 = E0 + E1 (horizontal pre-tap for Z), split DVE / GpSimd;
            # consumed a full step later, so no timing pressure here
            A1 = apool.tile([128, WH + 3], F32R, tag="A1")
            wA = self.PW + 2
            sp = (wA * 3) // 5
            nc.vector.tensor_add(
                out=A1[:rE, :sp], in0=E[:rE, :sp], in1=E[:rE, 1 : 1 + sp]
            )
            nc.gpsimd.tensor_add(
                out=A1[:rE, sp:wA], in0=E[:rE, sp:wA], in1=E[:rE, sp + 1 : wA + 1]
            )
            self.A1 = A1

        def p2(self):
            rE, h = self.rE, self.h
            E, X, A1 = self.E, self.X, self.A1
            # Z needs only 2 shifted matmuls: BT@A1 + BT@E2
            U = upool.tile([128, WH + 2], F32R, tag="U")
            ch = _chunks(self.PW + 2, self.CW)
            for idx, (cs, cl) in enumerate(ch):
                pz = ps_z.tile([128, C], F32, tag="pz")
                nc.tensor.matmul(
                    pz[:rE, :cl],
                    self.bt[:rE, :rE],
                    A1[:rE, cs : cs + cl],
                    start=True,
                    stop=False,
                )
                nc.tensor.matmul(
                    pz[:rE, :cl],
                    self.bt[:rE, :rE],
                    E[:rE, cs + 2 : cs + 2 + cl],
                    start=False,
                    stop=True,
                )
                Rz = rzpool.tile([128, C], F32, tag="Rz")
                nc.vector.reciprocal_approx_fast(out=Rz[:rE, :cl], in_=pz[:rE, :cl])
                nc.gpsimd.tensor_mul(
                    out=U[:rE, cs : cs + cl],
                    in0=X[:rE, cs + 2 : cs + 2 + cl],
                    in1=Rz[:rE, :cl],
                )
                # U at global-edge pad columns is 0 (fold drops OOB)
                if idx == 0 and h == 0:
                    up = U[:rE, 0:1] if self.kind == "n" else U[0:32, 0:1]
                    nc.gpsimd.memset(up.bitcast(F32), 0.0)
                if idx == len(ch) - 1 and h == WS - 1:
                    w = self.PW
                    up = (U[:rE, w + 1 : w + 2] if self.kind == "n"
                          else U[96:128, w + 1 : w + 2])
                    nc.gpsimd.memset(up.bitcast(F32), 0.0)
            self.U = U

        def p3(self):
            o, R, g0 = self.o, self.R, self.g0
            rE, rS = self.rE, self.rS
            E, U = self.E, self.U
            res = respool.tile([128, WH], F32, tag="res")
            for cs, cl in _chunks(self.PW, self.CW):
                ps = ps_s.tile([128, C], F32, tag="ps")
                for v in range(3):
                    nc.tensor.matmul(
                        ps[:rS, :cl],
                        self.bb[:rE, :rS],
                        U[:rE, cs + v : cs + v + cl],
                        start=(v == 0),
                        stop=(v == 2),
                    )
                nc.vector.tensor_mul(
                    out=res[:rS, cs : cs + cl],
                    in0=E[:rS, cs + 2 : cs + 2 + cl],
                    in1=ps[:rS, :cl],
                )
            if self.kind == "n":
                nc.sync.dma_start(
                    out=out_d[o : o + R, g0 : g0 + WH], in_=res[2 : R + 2, :WH]
                )
            else:
                for b in range(4):
                    nc.sync.dma_start(
                        out=out_d[o : o + R, g0 + b * SEGW : g0 + (b + 1) * SEGW],
                        in_=res[32 * b + 2 : 32 * b + 2 + R, :SEGW],
                    )

    of, Rf = tiles[-1]
    units = []
    for o, R in tiles[:-1]:
        for h in range(WS):
            units.append(Unit("n", o, R, h))
    units.insert(1, Unit("f", of, Rf, 0))
    units.append(Unit("f", of, Rf, WS - 1))
    units[-1].CW = 128  # short serial chain at the drain

    LAG2, LAG3 = 1, 2
    n = len(units)
    units[0].p1_dma()
    for i in range(n + LAG3):
        if i + 1 < n:
            units[i + 1].p1_dma()
        if i == 0:
            nc.sync.dma_start(
                out=bigb[:, 5 * 128 :].rearrange("p (i m) -> p i m", i=5),
                in_=bands_r[5:10].rearrange("i p m -> p i m"),
            )
        if 0 <= i - LAG2 < n:
            units[i - LAG2].p2()
        if 0 <= i - LAG3 < n:
            units[i - LAG3].p3()
        if i < n:
            units[i].p1()


_CACHE: dict = {}


def _build():
    if "nc" in _CACHE:
        return _CACHE["nc"]
    nc = bacc.Bacc(
        "TRN2", target_bir_lowering=False, debug=False, num_devices=N_CORES
    )
    xh_d = nc.dram_tensor(
        "xh", (RC + 2 * HALO + 2, W + 2 * HALO), F32, kind="ExternalInput"
    ).ap()
    mask_d = nc.dram_tensor("mask", (RC + 2 * HALO, 1), F32, kind="ExternalInput").ap()
    maskf_d = nc.dram_tensor("maskf", (128, 1), F32, kind="ExternalInput").ap()
    bands_d = nc.dram_tensor("bands", (10, 128, 128), F32, kind="ExternalInput").ap()
    out_d = nc.dram_tensor("out", (RC, W), F32, kind="ExternalOutput").ap()
    with tile.TileContext(nc) as tc:
        _energy_body(tc, out_d, xh_d, mask_d, maskf_d, bands_d)
    nc.compile()
    _CACHE["nc"] = nc
    return nc


def kernel(shareable_energy: np.ndarray, kernel: np.ndarray, **_run_kw) -> np.ndarray:
    x = np.ascontiguousarray(np.asarray(shareable_energy, np.float32))
    k = np.asarray(kernel, np.float32)
    assert x.shape == (H, W), x.shape
    nc = _build()
    bands = _make_bands(k)
    in_maps = [_make_core_inputs(x, bands, core) for core in range(N_CORES)]
    r = run_bass_kernel_spmd(nc, in_maps, core_ids=list(range(N_CORES)), **_run_kw)
    out = np.concatenate([res["out"] for res in r.results], axis=0)
    if _run_kw:
        _CACHE["last_result"] = r
    return out

